# revision 6
# baseline (speedup 1.0000x reference)
"""AsymDCT Trainium2 kernel.

Computes, for x (16, 64, 224, 224) f32:
  x_low  (16, 64, 112, 112) — 8x8 block DCT, keep low 4x4 coeffs, inverse with 4x4 DCT
  x_high (16, 64, 224, 224) — x minus the low-frequency reconstruction

Decomposition (per 224x224 image X, all matrices block-diagonal, exact):
  T4 = T[:4]  (4,8);  P8 = T4^T T4;  E = t^T T4
  phase1:  M2[c,r] = sum_k X[k,c] * BT4^T[k,r]        (stationary = X chunks)
  phase2:  [Q | L2] = M2^T @ [BP^T | BE^T]            (stationary = M2 chunks)
  phase3:  V = B' @ Q;  x_high = X - V   (B' = blockdiag(T4^T))
           x_low = Bt' @ L2              (Bt' = blockdiag(t^T))
PE compute in bf16 (fp32 PSUM accumulation); in/out f32; the x_high
subtraction reads the original f32 input.

Data parallel: 1024 images sharded 128 per NeuronCore across 8 cores.
Host pre-permutes each shard into group-major layout so every device DMA
is one fully contiguous 2D block; outputs are inverse-permuted on host.
"""

import sys

import numpy as np

for _p in ("/opt/trn_rl_repo",):
    if _p not in sys.path:
        sys.path.insert(0, _p)

import ml_dtypes  # noqa: E402

import concourse.bass as bass  # noqa: E402
import concourse.mybir as mybir  # noqa: E402
from concourse import bacc  # noqa: E402
from concourse import tile  # noqa: E402
from concourse.bass_utils import run_bass_kernel_spmd  # noqa: E402

DT = mybir.dt
F32 = DT.float32
BF16 = DT.bfloat16

N_CORES = 8
BATCH, CH, IN = 16, 64, 224
LOW = 112
IMG_PER_CORE = (BATCH // N_CORES) * CH  # 128
G = 4  # images per group
N_GROUPS = IMG_PER_CORE // G  # 32

Alu = mybir.AluOpType


def dct_mat(N):
    n = np.arange(N)
    Tm = np.cos(np.pi * (2.0 * n[None, :] + 1.0) * n[:, None] / (2.0 * N))
    Tm[0, :] *= 1.0 / np.sqrt(2.0)
    Tm *= np.sqrt(2.0 / N)
    return Tm.astype(np.float32)


def build_weights(T, t):
    """Host-side block-diagonal weight matrices (fp64; cast at feed time)."""
    T = np.asarray(T, dtype=np.float64)
    t = np.asarray(t, dtype=np.float64)
    T4 = T[:4, :]
    P8 = T4.T @ T4
    E = t.T @ T4
    W4 = np.kron(np.eye(28), T4)  # (112, 224)
    w2a = np.kron(np.eye(16), P8)          # (128, 128)
    w2b = np.kron(np.eye(12), P8)          # (96, 96)
    w3a = np.kron(np.eye(16), E).T         # (128, 64)
    w3b = np.kron(np.eye(12), E).T         # (96, 48)
    return {
        "w1a": np.kron(np.eye(16), T4).T,   # (128, 64)
        "w1b": np.kron(np.eye(12), T4).T,   # (96, 48)
        "w23a": np.concatenate([w2a, w3a], axis=1),  # (128, 192)
        "w23b": np.concatenate([w2b, w3b], axis=1),  # (96, 144)
        "w4a": W4[:, :128],                 # (112, 128)
        "w4b": W4[:, 128:],                 # (112, 96)
        "w5": np.kron(np.eye(28), t),       # (112, 112)
    }


W_SHAPES = {
    "w1a": (128, 64), "w1b": (96, 48),
    "w23a": (128, 192), "w23b": (96, 144),
    "w4a": (112, 128), "w4b": (112, 96),
    "w5": (112, 112),
}


def build_nc():
    nc = bacc.Bacc("TRN2", target_bir_lowering=False, debug=False,
                   num_devices=N_CORES)

    # Group-major host-prepared layouts: each DMA is one contiguous block.
    xin_t = nc.declare_dram_parameter("xt", [N_GROUPS, 128, G * IN], F32,
                                      isOutput=False)
    xin_b = nc.declare_dram_parameter("xb", [N_GROUPS, 96, G * IN], F32,
                                      isOutput=False)
    wd = {
        k: nc.declare_dram_parameter(k, list(sh), BF16, isOutput=False)
        for k, sh in W_SHAPES.items()
    }
    xlow_d = nc.declare_dram_parameter("xlow", [N_GROUPS, LOW, G * LOW], F32,
                                       isOutput=True)
    xhigh_t = nc.declare_dram_parameter("xht", [N_GROUPS, 128, G * IN], F32,
                                        isOutput=True)
    xhigh_b = nc.declare_dram_parameter("xhb", [N_GROUPS, 96, G * IN], F32,
                                        isOutput=True)

    with tile.TileContext(nc) as tc:
        with (
            tc.tile_pool(name="w", bufs=1) as wpool,
            tc.tile_pool(name="xf", bufs=3) as xfpool,
            tc.tile_pool(name="xi", bufs=3) as xpool,
            tc.tile_pool(name="m2", bufs=3) as m2pool,
            tc.tile_pool(name="qs", bufs=3) as qspool,
            tc.tile_pool(name="xh", bufs=3) as xhpool,
            tc.tile_pool(name="xls", bufs=3) as xlspool,
            tc.tile_pool(name="pp1", bufs=1, space="PSUM") as pp1,
            tc.tile_pool(name="ppq", bufs=1, space="PSUM") as ppq,
            tc.tile_pool(name="ppv", bufs=1, space="PSUM") as ppv,
            tc.tile_pool(name="ppxl", bufs=1, space="PSUM") as ppxl,
        ):
            w = {}
            for k, sh in W_SHAPES.items():
                w[k] = wpool.tile(list(sh), BF16, tag=k, name=k)
                nc.sync.dma_start(w[k][:], wd[k][:])

            for g in range(N_GROUPS):
                # ---- load f32 (HWDGE, contiguous), cast to bf16 on GPSIMD --
                xtf = xfpool.tile([128, G * IN], F32, tag="xtf")
                xbf = xfpool.tile([96, G * IN], F32, tag="xbf")
                nc.sync.dma_start(xtf[:], xin_t[g])
                nc.sync.dma_start(xbf[:], xin_b[g])
                xt = xpool.tile([128, G * IN], BF16, tag="xt")
                xb = xpool.tile([96, G * IN], BF16, tag="xb")
                nc.gpsimd.tensor_copy(xt[:], xtf[:])
                nc.gpsimd.tensor_copy(xb[:], xbf[:])

                # ---- phase 1: M2 (224, G*112) in PSUM ----
                p1t = pp1.tile([128, G * LOW], F32, tag="p1t")
                p1b = pp1.tile([96, G * LOW], F32, tag="p1b")
                for j in range(G):
                    xc = j * IN
                    mc = j * LOW
                    nc.tensor.matmul(p1t[:, mc:mc + 64],
                                     xt[:, xc:xc + 128], w["w1a"][:])
                    nc.tensor.matmul(p1b[:, mc:mc + 64],
                                     xt[:, xc + 128:xc + 224], w["w1a"][:])
                    nc.tensor.matmul(p1t[:, mc + 64:mc + 112],
                                     xb[:, xc:xc + 128], w["w1b"][:])
                    nc.tensor.matmul(p1b[:, mc + 64:mc + 112],
                                     xb[:, xc + 128:xc + 224], w["w1b"][:])

                m2t = m2pool.tile([128, G * LOW], BF16, tag="m2t")
                m2b = m2pool.tile([96, G * LOW], BF16, tag="m2b")
                nc.scalar.copy(m2t[:], p1t[:])
                nc.scalar.copy(m2b[:], p1b[:])

                # ---- phase 2: per image [Q | L2] fused matmuls ----
                # qsa: per image [QL(128) | L2L(64)]; qsb: [QR(96) | L2R(48)]
                qsa = qspool.tile([112, G * 192], BF16, tag="qsa")
                qsb = qspool.tile([112, G * 144], BF16, tag="qsb")
                xht = xhpool.tile([128, G * IN], F32, tag="xht")
                xhb = xhpool.tile([96, G * IN], F32, tag="xhb")

                for pp in range(G // 2):
                    qla = ppq.tile([112, 2 * 192], F32, tag="qla", bufs=2)
                    qlb = ppq.tile([112, 2 * 144], F32, tag="qlb")
                    for jj in range(2):
                        j = pp * 2 + jj
                        mc = j * LOW
                        nc.tensor.matmul(qla[:, jj * 192:(jj + 1) * 192],
                                         m2t[:, mc:mc + 112], w["w23a"][:])
                        nc.tensor.matmul(qlb[:, jj * 144:(jj + 1) * 144],
                                         m2b[:, mc:mc + 112], w["w23b"][:])
                    nc.scalar.copy(qsa[:, pp * 384:(pp + 1) * 384], qla[:])
                    nc.scalar.copy(qsb[:, pp * 288:(pp + 1) * 288], qlb[:])

                    # ---- phase 3 high: V = B' @ Q; x_high = X - V ----
                    vt = ppv.tile([128, 2 * IN], F32, tag="vt")
                    vb = ppv.tile([96, 2 * IN], F32, tag="vb")
                    # Q columns of the two images, strided over qsa/qsb
                    rqa = qsa[:, pp * 384:(pp + 1) * 384].rearrange(
                        "p (i c) -> p i c", i=2)[:, :, 0:128]
                    rqb = qsb[:, pp * 288:(pp + 1) * 288].rearrange(
                        "p (i c) -> p i c", i=2)[:, :, 0:96]
                    vt3 = vt[:].rearrange("p (i c) -> p i c", i=2)
                    vb3 = vb[:].rearrange("p (i c) -> p i c", i=2)
                    nc.tensor.matmul(vt3[:, :, 0:128], w["w4a"][:], rqa)
                    nc.tensor.matmul(vt3[:, :, 128:224], w["w4a"][:], rqb)
                    nc.tensor.matmul(vb3[:, :, 0:128], w["w4b"][:], rqa)
                    nc.tensor.matmul(vb3[:, :, 128:224], w["w4b"][:], rqb)
                    pq = pp * 2 * IN
                    nc.vector.scalar_tensor_tensor(
                        xht[:, pq:pq + 2 * IN], xtf[:, pq:pq + 2 * IN], 1.0,
                        vt[:], Alu.mult, Alu.subtract)
                    nc.vector.scalar_tensor_tensor(
                        xhb[:, pq:pq + 2 * IN], xbf[:, pq:pq + 2 * IN], 1.0,
                        vb[:], Alu.mult, Alu.subtract)

                # ---- phase 3 low: x_low = Bt' @ L2 (strided rhs) ----
                xlp = ppxl.tile([112, G * LOW], F32, tag="xlp")
                la = qsa[:].rearrange("p (i c) -> p i c", i=G)[:, :, 128:192]
                lb = qsb[:].rearrange("p (i c) -> p i c", i=G)[:, :, 96:144]
                xlp3 = xlp[:].rearrange("p (i c) -> p i c", i=G)
                nc.tensor.matmul(xlp3[:, :, 0:64], w["w5"][:], la)
                nc.tensor.matmul(xlp3[:, :, 64:112], w["w5"][:], lb)
                xls = xlspool.tile([112, G * LOW], F32, tag="xls")
                nc.scalar.copy(xls[:], xlp[:])

                # ---- stores (contiguous blocks) ----
                nc.sync.dma_start(xhigh_t[g], xht[:])
                nc.sync.dma_start(xhigh_b[g], xhb[:])
                nc.sync.dma_start(xlow_d[g], xls[:])

    nc.compile()
    return nc


_NC_CACHE = None


def _get_nc():
    global _NC_CACHE
    if _NC_CACHE is None:
        _NC_CACHE = build_nc()
    return _NC_CACHE


def _shard_layout(shard):
    """(128, 224, 224) f32 -> group-major contiguous blocks."""
    grp = shard.reshape(N_GROUPS, G, IN, IN)
    xt = np.ascontiguousarray(
        grp[:, :, 0:128, :].transpose(0, 2, 1, 3)).reshape(N_GROUPS, 128,
                                                           G * IN)
    xb = np.ascontiguousarray(
        grp[:, :, 128:224, :].transpose(0, 2, 1, 3)).reshape(N_GROUPS, 96,
                                                             G * IN)
    return xt, xb


def _unshard_high(xht, xhb):
    """Inverse of _shard_layout for the x_high output."""
    t = xht.reshape(N_GROUPS, 128, G, IN).transpose(0, 2, 1, 3)
    b = xhb.reshape(N_GROUPS, 96, G, IN).transpose(0, 2, 1, 3)
    return np.concatenate([t, b], axis=2).reshape(IMG_PER_CORE, IN, IN)


def _unshard_low(xlo):
    lo = xlo.reshape(N_GROUPS, LOW, G, LOW).transpose(0, 2, 1, 3)
    return lo.reshape(IMG_PER_CORE, LOW, LOW)


def run(x, T=None, t=None, trace=False):
    x = np.ascontiguousarray(np.asarray(x, dtype=np.float32))
    assert x.shape == (BATCH, CH, IN, IN), x.shape
    if T is None:
        T = dct_mat(8)
    if t is None:
        t = dct_mat(4)
    weights = build_weights(T, t)
    wmaps = {k: np.ascontiguousarray(v.astype(ml_dtypes.bfloat16))
             for k, v in weights.items()}

    per_core = BATCH // N_CORES  # 2
    in_maps = []
    for i in range(N_CORES):
        shard = x[i * per_core:(i + 1) * per_core].reshape(IMG_PER_CORE, IN, IN)
        xt, xb = _shard_layout(shard)
        in_maps.append({"xt": xt, "xb": xb, **wmaps})

    nc = _get_nc()
    res = run_bass_kernel_spmd(nc, in_maps, core_ids=list(range(N_CORES)),
                               trace=trace)
    xlow = np.stack([
        _unshard_low(np.asarray(res.results[i]["xlow"]))
        for i in range(N_CORES)
    ]).reshape(BATCH, CH, LOW, LOW)
    xhigh = np.stack([
        _unshard_high(np.asarray(res.results[i]["xht"]),
                      np.asarray(res.results[i]["xhb"]))
        for i in range(N_CORES)
    ]).reshape(BATCH, CH, IN, IN)
    return (xlow, xhigh), res


def kernel(x, T=None, t=None):
    (xlow, xhigh), _ = run(x, T, t, trace=False)
    return (xlow, xhigh)


if __name__ == "__main__":
    nc = build_nc()
    print("built ok")


# revision 7
# speedup vs baseline: 1.0001x; 1.0001x over previous
"""AsymDCT Trainium2 kernel.

Computes, for x (16, 64, 224, 224) f32:
  x_low  (16, 64, 112, 112) — 8x8 block DCT, keep low 4x4 coeffs, inverse with 4x4 DCT
  x_high (16, 64, 224, 224) — x minus the low-frequency reconstruction

Decomposition (per 224x224 image X, all matrices block-diagonal, exact):
  T4 = T[:4]  (4,8);  P8 = T4^T T4;  E = t^T T4
  phase1:  M2[c,r] = sum_k X[k,c] * BT4^T[k,r]        (stationary = X chunks)
  phase2:  [Q | L2] = M2^T @ [BP^T | BE^T]            (stationary = M2 chunks)
  phase3:  V = B' @ Q;  x_high = X - V   (B' = blockdiag(T4^T))
           x_low = Bt' @ L2              (Bt' = blockdiag(t^T))
PE compute in bf16 (fp32 PSUM accumulation); in/out f32; the x_high
subtraction reads the original f32 input.

Data parallel: 1024 images sharded 128 per NeuronCore across 8 cores.
Host pre-permutes each shard into group-major layout so every device DMA
is one fully contiguous 2D block; outputs are inverse-permuted on host.
"""

import sys

import numpy as np

for _p in ("/opt/trn_rl_repo",):
    if _p not in sys.path:
        sys.path.insert(0, _p)

import ml_dtypes  # noqa: E402

import concourse.bass as bass  # noqa: E402
import concourse.mybir as mybir  # noqa: E402
from concourse import bacc  # noqa: E402
from concourse import tile  # noqa: E402
from concourse.bass_utils import run_bass_kernel_spmd  # noqa: E402

DT = mybir.dt
F32 = DT.float32
BF16 = DT.bfloat16

N_CORES = 8
BATCH, CH, IN = 16, 64, 224
LOW = 112
IMG_PER_CORE = (BATCH // N_CORES) * CH  # 128
G = 4  # images per group
N_GROUPS = IMG_PER_CORE // G  # 32

Alu = mybir.AluOpType


def dct_mat(N):
    n = np.arange(N)
    Tm = np.cos(np.pi * (2.0 * n[None, :] + 1.0) * n[:, None] / (2.0 * N))
    Tm[0, :] *= 1.0 / np.sqrt(2.0)
    Tm *= np.sqrt(2.0 / N)
    return Tm.astype(np.float32)


def build_weights(T, t):
    """Host-side block-diagonal weight matrices (fp64; cast at feed time)."""
    T = np.asarray(T, dtype=np.float64)
    t = np.asarray(t, dtype=np.float64)
    T4 = T[:4, :]
    P8 = T4.T @ T4
    E = t.T @ T4
    W4 = np.kron(np.eye(28), T4)  # (112, 224)
    w2a = np.kron(np.eye(16), P8)          # (128, 128)
    w2b = np.kron(np.eye(12), P8)          # (96, 96)
    w3a = np.kron(np.eye(16), E).T         # (128, 64)
    w3b = np.kron(np.eye(12), E).T         # (96, 48)
    return {
        "w1a": np.kron(np.eye(16), T4).T,   # (128, 64)
        "w1b": np.kron(np.eye(12), T4).T,   # (96, 48)
        "w23a": np.concatenate([w2a, w3a], axis=1),  # (128, 192)
        "w23b": np.concatenate([w2b, w3b], axis=1),  # (96, 144)
        "w4a": W4[:, :128],                 # (112, 128)
        "w4b": W4[:, 128:],                 # (112, 96)
        "w5": np.kron(np.eye(28), t),       # (112, 112)
    }


W_SHAPES = {
    "w1a": (128, 64), "w1b": (96, 48),
    "w23a": (128, 192), "w23b": (96, 144),
    "w4a": (112, 128), "w4b": (112, 96),
    "w5": (112, 112),
}


def build_nc():
    nc = bacc.Bacc("TRN2", target_bir_lowering=False, debug=False,
                   num_devices=N_CORES)

    # Group-major host-prepared layouts: each DMA is one contiguous block.
    xin_t = nc.declare_dram_parameter("xt", [N_GROUPS, 128, G * IN], F32,
                                      isOutput=False)
    xin_b = nc.declare_dram_parameter("xb", [N_GROUPS, 96, G * IN], F32,
                                      isOutput=False)
    wd = {
        k: nc.declare_dram_parameter(k, list(sh), BF16, isOutput=False)
        for k, sh in W_SHAPES.items()
    }
    xlow_d = nc.declare_dram_parameter("xlow", [N_GROUPS, LOW, G * LOW], F32,
                                       isOutput=True)
    xhigh_t = nc.declare_dram_parameter("xht", [N_GROUPS, 128, G * IN], F32,
                                        isOutput=True)
    xhigh_b = nc.declare_dram_parameter("xhb", [N_GROUPS, 96, G * IN], F32,
                                        isOutput=True)

    with tile.TileContext(nc) as tc:
        with (
            tc.tile_pool(name="w", bufs=1) as wpool,
            tc.tile_pool(name="xf", bufs=3) as xfpool,
            tc.tile_pool(name="xi", bufs=3) as xpool,
            tc.tile_pool(name="m2", bufs=3) as m2pool,
            tc.tile_pool(name="qs", bufs=3) as qspool,
            tc.tile_pool(name="xh", bufs=3) as xhpool,
            tc.tile_pool(name="xls", bufs=3) as xlspool,
            tc.tile_pool(name="pp1", bufs=1, space="PSUM") as pp1,
            tc.tile_pool(name="ppq", bufs=1, space="PSUM") as ppq,
            tc.tile_pool(name="ppv", bufs=1, space="PSUM") as ppv,
            tc.tile_pool(name="ppxl", bufs=1, space="PSUM") as ppxl,
        ):
            w = {}
            for k, sh in W_SHAPES.items():
                w[k] = wpool.tile(list(sh), BF16, tag=k, name=k)
                nc.sync.dma_start(w[k][:], wd[k][:])

            for g in range(N_GROUPS):
                # ---- load f32 (HWDGE, contiguous), cast to bf16 on GPSIMD --
                xtf = xfpool.tile([128, G * IN], F32, tag="xtf")
                xbf = xfpool.tile([96, G * IN], F32, tag="xbf")
                nc.sync.dma_start(xtf[:], xin_t[g])
                nc.sync.dma_start(xbf[:], xin_b[g])
                xt = xpool.tile([128, G * IN], BF16, tag="xt")
                xb = xpool.tile([96, G * IN], BF16, tag="xb")
                nc.vector.tensor_copy(xt[:], xtf[:])
                nc.vector.tensor_copy(xb[:], xbf[:])

                # ---- phase 1: M2 (224, G*112) in PSUM ----
                p1t = pp1.tile([128, G * LOW], F32, tag="p1t")
                p1b = pp1.tile([96, G * LOW], F32, tag="p1b")
                for j in range(G):
                    xc = j * IN
                    mc = j * LOW
                    nc.tensor.matmul(p1t[:, mc:mc + 64],
                                     xt[:, xc:xc + 128], w["w1a"][:])
                    nc.tensor.matmul(p1b[:, mc:mc + 64],
                                     xt[:, xc + 128:xc + 224], w["w1a"][:])
                    nc.tensor.matmul(p1t[:, mc + 64:mc + 112],
                                     xb[:, xc:xc + 128], w["w1b"][:])
                    nc.tensor.matmul(p1b[:, mc + 64:mc + 112],
                                     xb[:, xc + 128:xc + 224], w["w1b"][:])

                m2t = m2pool.tile([128, G * LOW], BF16, tag="m2t")
                m2b = m2pool.tile([96, G * LOW], BF16, tag="m2b")
                nc.scalar.copy(m2t[:], p1t[:])
                nc.scalar.copy(m2b[:], p1b[:])

                # ---- phase 2: per image [Q | L2] fused matmuls ----
                # qsa: per image [QL(128) | L2L(64)]; qsb: [QR(96) | L2R(48)]
                qsa = qspool.tile([112, G * 192], BF16, tag="qsa")
                qsb = qspool.tile([112, G * 144], BF16, tag="qsb")
                xht = xhpool.tile([128, G * IN], F32, tag="xht")
                xhb = xhpool.tile([96, G * IN], F32, tag="xhb")

                for pp in range(G // 2):
                    qla = ppq.tile([112, 2 * 192], F32, tag="qla", bufs=2)
                    qlb = ppq.tile([112, 2 * 144], F32, tag="qlb")
                    for jj in range(2):
                        j = pp * 2 + jj
                        mc = j * LOW
                        nc.tensor.matmul(qla[:, jj * 192:(jj + 1) * 192],
                                         m2t[:, mc:mc + 112], w["w23a"][:])
                        nc.tensor.matmul(qlb[:, jj * 144:(jj + 1) * 144],
                                         m2b[:, mc:mc + 112], w["w23b"][:])
                    nc.scalar.copy(qsa[:, pp * 384:(pp + 1) * 384], qla[:])
                    nc.scalar.copy(qsb[:, pp * 288:(pp + 1) * 288], qlb[:])

                    # ---- phase 3 high: V = B' @ Q; x_high = X - V ----
                    vt = ppv.tile([128, 2 * IN], F32, tag="vt")
                    vb = ppv.tile([96, 2 * IN], F32, tag="vb")
                    # Q columns of the two images, strided over qsa/qsb
                    rqa = qsa[:, pp * 384:(pp + 1) * 384].rearrange(
                        "p (i c) -> p i c", i=2)[:, :, 0:128]
                    rqb = qsb[:, pp * 288:(pp + 1) * 288].rearrange(
                        "p (i c) -> p i c", i=2)[:, :, 0:96]
                    vt3 = vt[:].rearrange("p (i c) -> p i c", i=2)
                    vb3 = vb[:].rearrange("p (i c) -> p i c", i=2)
                    nc.tensor.matmul(vt3[:, :, 0:128], w["w4a"][:], rqa)
                    nc.tensor.matmul(vt3[:, :, 128:224], w["w4a"][:], rqb)
                    nc.tensor.matmul(vb3[:, :, 0:128], w["w4b"][:], rqa)
                    nc.tensor.matmul(vb3[:, :, 128:224], w["w4b"][:], rqb)
                    pq = pp * 2 * IN
                    nc.vector.scalar_tensor_tensor(
                        xht[:, pq:pq + 2 * IN], xtf[:, pq:pq + 2 * IN], 1.0,
                        vt[:], Alu.mult, Alu.subtract)
                    nc.vector.scalar_tensor_tensor(
                        xhb[:, pq:pq + 2 * IN], xbf[:, pq:pq + 2 * IN], 1.0,
                        vb[:], Alu.mult, Alu.subtract)

                # ---- phase 3 low: x_low = Bt' @ L2 (strided rhs) ----
                xlp = ppxl.tile([112, G * LOW], F32, tag="xlp")
                la = qsa[:].rearrange("p (i c) -> p i c", i=G)[:, :, 128:192]
                lb = qsb[:].rearrange("p (i c) -> p i c", i=G)[:, :, 96:144]
                xlp3 = xlp[:].rearrange("p (i c) -> p i c", i=G)
                nc.tensor.matmul(xlp3[:, :, 0:64], w["w5"][:], la)
                nc.tensor.matmul(xlp3[:, :, 64:112], w["w5"][:], lb)
                xls = xlspool.tile([112, G * LOW], F32, tag="xls")
                nc.scalar.copy(xls[:], xlp[:])

                # ---- stores (contiguous blocks) ----
                nc.scalar.dma_start(xhigh_t[g], xht[:])
                nc.scalar.dma_start(xhigh_b[g], xhb[:])
                nc.scalar.dma_start(xlow_d[g], xls[:])

    nc.compile()
    return nc


_NC_CACHE = None


def _get_nc():
    global _NC_CACHE
    if _NC_CACHE is None:
        _NC_CACHE = build_nc()
    return _NC_CACHE


def _shard_layout(shard):
    """(128, 224, 224) f32 -> group-major contiguous blocks."""
    grp = shard.reshape(N_GROUPS, G, IN, IN)
    xt = np.ascontiguousarray(
        grp[:, :, 0:128, :].transpose(0, 2, 1, 3)).reshape(N_GROUPS, 128,
                                                           G * IN)
    xb = np.ascontiguousarray(
        grp[:, :, 128:224, :].transpose(0, 2, 1, 3)).reshape(N_GROUPS, 96,
                                                             G * IN)
    return xt, xb


def _unshard_high(xht, xhb):
    """Inverse of _shard_layout for the x_high output."""
    t = xht.reshape(N_GROUPS, 128, G, IN).transpose(0, 2, 1, 3)
    b = xhb.reshape(N_GROUPS, 96, G, IN).transpose(0, 2, 1, 3)
    return np.concatenate([t, b], axis=2).reshape(IMG_PER_CORE, IN, IN)


def _unshard_low(xlo):
    lo = xlo.reshape(N_GROUPS, LOW, G, LOW).transpose(0, 2, 1, 3)
    return lo.reshape(IMG_PER_CORE, LOW, LOW)


def run(x, T=None, t=None, trace=False):
    x = np.ascontiguousarray(np.asarray(x, dtype=np.float32))
    assert x.shape == (BATCH, CH, IN, IN), x.shape
    if T is None:
        T = dct_mat(8)
    if t is None:
        t = dct_mat(4)
    weights = build_weights(T, t)
    wmaps = {k: np.ascontiguousarray(v.astype(ml_dtypes.bfloat16))
             for k, v in weights.items()}

    per_core = BATCH // N_CORES  # 2
    in_maps = []
    for i in range(N_CORES):
        shard = x[i * per_core:(i + 1) * per_core].reshape(IMG_PER_CORE, IN, IN)
        xt, xb = _shard_layout(shard)
        in_maps.append({"xt": xt, "xb": xb, **wmaps})

    nc = _get_nc()
    res = run_bass_kernel_spmd(nc, in_maps, core_ids=list(range(N_CORES)),
                               trace=trace)
    xlow = np.stack([
        _unshard_low(np.asarray(res.results[i]["xlow"]))
        for i in range(N_CORES)
    ]).reshape(BATCH, CH, LOW, LOW)
    xhigh = np.stack([
        _unshard_high(np.asarray(res.results[i]["xht"]),
                      np.asarray(res.results[i]["xhb"]))
        for i in range(N_CORES)
    ]).reshape(BATCH, CH, IN, IN)
    return (xlow, xhigh), res


def kernel(x, T=None, t=None):
    (xlow, xhigh), _ = run(x, T, t, trace=False)
    return (xlow, xhigh)


if __name__ == "__main__":
    nc = build_nc()
    print("built ok")


# revision 8
# speedup vs baseline: 1.4832x; 1.4831x over previous
"""AsymDCT Trainium2 kernel.

Computes, for x (16, 64, 224, 224) f32:
  x_low  (16, 64, 112, 112) — 8x8 block DCT, keep low 4x4 coeffs, inverse with 4x4 DCT
  x_high (16, 64, 224, 224) — x minus the low-frequency reconstruction

Decomposition (per 224x224 image X, all matrices block-diagonal, exact):
  T4 = T[:4]  (4,8);  P8 = T4^T T4;  E = t^T T4
  phase1:  M2[c,r] = sum_k X[k,c] * BT4^T[k,r]        (stationary = X chunks)
  phase2:  [Q | L2] = M2^T @ [BP^T | BE^T]            (stationary = M2 chunks)
  phase3:  V = B' @ Q;  x_high = X - V   (B' = blockdiag(T4^T))
           x_low = Bt' @ L2              (Bt' = blockdiag(t^T))
PE compute in bf16 (fp32 PSUM accumulation); in/out f32; the x_high
subtraction reads the original f32 input.

Data parallel: 1024 images sharded 128 per NeuronCore across 8 cores.
Host pre-permutes each shard into group-major layout so every device DMA
is one fully contiguous 2D block; outputs are inverse-permuted on host.
"""

import sys

import numpy as np

for _p in ("/opt/trn_rl_repo",):
    if _p not in sys.path:
        sys.path.insert(0, _p)

import ml_dtypes  # noqa: E402

import concourse.bass as bass  # noqa: E402
import concourse.mybir as mybir  # noqa: E402
from concourse import bacc  # noqa: E402
from concourse import tile  # noqa: E402
from concourse.bass_utils import run_bass_kernel_spmd  # noqa: E402

DT = mybir.dt
F32 = DT.float32
BF16 = DT.bfloat16

N_CORES = 8
BATCH, CH, IN = 16, 64, 224
LOW = 112
IMG_PER_CORE = (BATCH // N_CORES) * CH  # 128
G = 4  # images per group
N_GROUPS = IMG_PER_CORE // G  # 32

Alu = mybir.AluOpType


def dct_mat(N):
    n = np.arange(N)
    Tm = np.cos(np.pi * (2.0 * n[None, :] + 1.0) * n[:, None] / (2.0 * N))
    Tm[0, :] *= 1.0 / np.sqrt(2.0)
    Tm *= np.sqrt(2.0 / N)
    return Tm.astype(np.float32)


def build_weights(T, t):
    """Host-side block-diagonal weight matrices (fp64; cast at feed time)."""
    T = np.asarray(T, dtype=np.float64)
    t = np.asarray(t, dtype=np.float64)
    T4 = T[:4, :]
    P8 = T4.T @ T4
    E = t.T @ T4
    W4 = np.kron(np.eye(28), T4)  # (112, 224)
    w2a = np.kron(np.eye(16), P8)          # (128, 128)
    w2b = np.kron(np.eye(12), P8)          # (96, 96)
    w3a = np.kron(np.eye(16), E).T         # (128, 64)
    w3b = np.kron(np.eye(12), E).T         # (96, 48)
    return {
        "w1a": np.kron(np.eye(16), T4).T,   # (128, 64)
        "w1b": np.kron(np.eye(12), T4).T,   # (96, 48)
        "w23a": np.concatenate([w2a, w3a], axis=1),  # (128, 192)
        "w23b": np.concatenate([w2b, w3b], axis=1),  # (96, 144)
        "w4a": W4[:, :128],                 # (112, 128)
        "w4b": W4[:, 128:],                 # (112, 96)
        "w5": np.kron(np.eye(28), t),       # (112, 112)
    }


W_SHAPES = {
    "w1a": (128, 64), "w1b": (96, 48),
    "w23a": (128, 192), "w23b": (96, 144),
    "w4a": (112, 128), "w4b": (112, 96),
    "w5": (112, 112),
}


def build_nc():
    nc = bacc.Bacc("TRN2", target_bir_lowering=False, debug=False,
                   num_devices=N_CORES)

    # Group-major host-prepared layouts: each DMA is one contiguous block.
    xin_t = nc.declare_dram_parameter("xt", [N_GROUPS, 128, G * IN], F32,
                                      isOutput=False)
    xin_b = nc.declare_dram_parameter("xb", [N_GROUPS, 96, G * IN], F32,
                                      isOutput=False)
    wd = {
        k: nc.declare_dram_parameter(k, list(sh), BF16, isOutput=False)
        for k, sh in W_SHAPES.items()
    }
    xlow_d = nc.declare_dram_parameter("xlow", [N_GROUPS, LOW, G * LOW], F32,
                                       isOutput=True)
    xhigh_t = nc.declare_dram_parameter("xht", [N_GROUPS, 128, G * IN], F32,
                                        isOutput=True)
    xhigh_b = nc.declare_dram_parameter("xhb", [N_GROUPS, 96, G * IN], F32,
                                        isOutput=True)

    with tile.TileContext(nc) as tc:
        with (
            tc.tile_pool(name="w", bufs=1) as wpool,
            tc.tile_pool(name="xf", bufs=3) as xfpool,
            tc.tile_pool(name="xi", bufs=3) as xpool,
            tc.tile_pool(name="m2", bufs=3) as m2pool,
            tc.tile_pool(name="qs", bufs=3) as qspool,
            tc.tile_pool(name="xh", bufs=3) as xhpool,
            tc.tile_pool(name="xls", bufs=3) as xlspool,
            tc.tile_pool(name="pp1", bufs=1, space="PSUM") as pp1,
            tc.tile_pool(name="ppq", bufs=1, space="PSUM") as ppq,
            tc.tile_pool(name="ppv", bufs=1, space="PSUM") as ppv,
            tc.tile_pool(name="ppxl", bufs=1, space="PSUM") as ppxl,
        ):
            w = {}
            for k, sh in W_SHAPES.items():
                w[k] = wpool.tile(list(sh), BF16, tag=k, name=k)
                nc.sync.dma_start(w[k][:], wd[k][:])

            for g in range(N_GROUPS):
                # ---- load f32 (HWDGE, contiguous), cast to bf16 on GPSIMD --
                xtf = xfpool.tile([128, G * IN], F32, tag="xtf")
                xbf = xfpool.tile([96, G * IN], F32, tag="xbf")
                nc.sync.dma_start(xtf[:], xin_t[g])
                nc.sync.dma_start(xbf[:], xin_b[g])
                xt = xpool.tile([128, G * IN], BF16, tag="xt")
                xb = xpool.tile([96, G * IN], BF16, tag="xb")
                nc.vector.tensor_copy(xt[:], xtf[:])
                nc.vector.tensor_copy(xb[:], xbf[:])

                # ---- phase 1: M2 (224, G*112) in PSUM ----
                p1t = pp1.tile([128, G * LOW], F32, tag="p1t")
                p1b = pp1.tile([96, G * LOW], F32, tag="p1b")
                for j in range(G):
                    xc = j * IN
                    mc = j * LOW
                    nc.tensor.matmul(p1t[:, mc:mc + 64],
                                     xt[:, xc:xc + 128], w["w1a"][:])
                    nc.tensor.matmul(p1b[:, mc:mc + 64],
                                     xt[:, xc + 128:xc + 224], w["w1a"][:])
                    nc.tensor.matmul(p1t[:, mc + 64:mc + 112],
                                     xb[:, xc:xc + 128], w["w1b"][:])
                    nc.tensor.matmul(p1b[:, mc + 64:mc + 112],
                                     xb[:, xc + 128:xc + 224], w["w1b"][:])

                m2t = m2pool.tile([128, G * LOW], BF16, tag="m2t")
                m2b = m2pool.tile([96, G * LOW], BF16, tag="m2b")
                for pp in range(G // 2):
                    h = pp * 2 * LOW
                    nc.scalar.copy(m2t[:, h:h + 2 * LOW], p1t[:, h:h + 2 * LOW])
                    nc.scalar.copy(m2b[:, h:h + 2 * LOW], p1b[:, h:h + 2 * LOW])

                # ---- phase 2: per image [Q | L2] fused matmuls ----
                # qsa: per image [QL(128) | L2L(64)]; qsb: [QR(96) | L2R(48)]
                qsa = qspool.tile([112, G * 192], BF16, tag="qsa")
                qsb = qspool.tile([112, G * 144], BF16, tag="qsb")
                xht = xhpool.tile([128, G * IN], F32, tag="xht")
                xhb = xhpool.tile([96, G * IN], F32, tag="xhb")

                for pp in range(G // 2):
                    qla = ppq.tile([112, 2 * 192], F32, tag="qla", bufs=2)
                    qlb = ppq.tile([112, 2 * 144], F32, tag="qlb")
                    for jj in range(2):
                        j = pp * 2 + jj
                        mc = j * LOW
                        nc.tensor.matmul(qla[:, jj * 192:(jj + 1) * 192],
                                         m2t[:, mc:mc + 112], w["w23a"][:])
                        nc.tensor.matmul(qlb[:, jj * 144:(jj + 1) * 144],
                                         m2b[:, mc:mc + 112], w["w23b"][:])
                    nc.scalar.copy(qsa[:, pp * 384:(pp + 1) * 384], qla[:])
                    nc.scalar.copy(qsb[:, pp * 288:(pp + 1) * 288], qlb[:])

                    # ---- phase 3 high: V = B' @ Q; x_high = X - V ----
                    vt = ppv.tile([128, 2 * IN], F32, tag="vt")
                    vb = ppv.tile([96, 2 * IN], F32, tag="vb")
                    # Q columns of the two images, strided over qsa/qsb
                    rqa = qsa[:, pp * 384:(pp + 1) * 384].rearrange(
                        "p (i c) -> p i c", i=2)[:, :, 0:128]
                    rqb = qsb[:, pp * 288:(pp + 1) * 288].rearrange(
                        "p (i c) -> p i c", i=2)[:, :, 0:96]
                    vt3 = vt[:].rearrange("p (i c) -> p i c", i=2)
                    vb3 = vb[:].rearrange("p (i c) -> p i c", i=2)
                    nc.tensor.matmul(vt3[:, :, 0:128], w["w4a"][:], rqa)
                    nc.tensor.matmul(vt3[:, :, 128:224], w["w4a"][:], rqb)
                    nc.tensor.matmul(vb3[:, :, 0:128], w["w4b"][:], rqa)
                    nc.tensor.matmul(vb3[:, :, 128:224], w["w4b"][:], rqb)
                    pq = pp * 2 * IN
                    nc.vector.scalar_tensor_tensor(
                        xht[:, pq:pq + 2 * IN], xtf[:, pq:pq + 2 * IN], 1.0,
                        vt[:], Alu.mult, Alu.subtract)
                    nc.vector.scalar_tensor_tensor(
                        xhb[:, pq:pq + 2 * IN], xbf[:, pq:pq + 2 * IN], 1.0,
                        vb[:], Alu.mult, Alu.subtract)

                # ---- phase 3 low: x_low = Bt' @ L2 (strided rhs) ----
                xlp = ppxl.tile([112, G * LOW], F32, tag="xlp")
                la = qsa[:].rearrange("p (i c) -> p i c", i=G)[:, :, 128:192]
                lb = qsb[:].rearrange("p (i c) -> p i c", i=G)[:, :, 96:144]
                xlp3 = xlp[:].rearrange("p (i c) -> p i c", i=G)
                nc.tensor.matmul(xlp3[:, :, 0:64], w["w5"][:], la)
                nc.tensor.matmul(xlp3[:, :, 64:112], w["w5"][:], lb)
                xls = xlspool.tile([112, G * LOW], F32, tag="xls")
                nc.scalar.copy(xls[:], xlp[:])

                # ---- stores (contiguous blocks) ----
                nc.gpsimd.dma_start(xhigh_t[g], xht[:])
                nc.gpsimd.dma_start(xhigh_b[g], xhb[:])
                nc.sync.dma_start(xlow_d[g], xls[:])

    nc.compile()
    return nc


_NC_CACHE = None


def _get_nc():
    global _NC_CACHE
    if _NC_CACHE is None:
        _NC_CACHE = build_nc()
    return _NC_CACHE


def _shard_layout(shard):
    """(128, 224, 224) f32 -> group-major contiguous blocks."""
    grp = shard.reshape(N_GROUPS, G, IN, IN)
    xt = np.ascontiguousarray(
        grp[:, :, 0:128, :].transpose(0, 2, 1, 3)).reshape(N_GROUPS, 128,
                                                           G * IN)
    xb = np.ascontiguousarray(
        grp[:, :, 128:224, :].transpose(0, 2, 1, 3)).reshape(N_GROUPS, 96,
                                                             G * IN)
    return xt, xb


def _unshard_high(xht, xhb):
    """Inverse of _shard_layout for the x_high output."""
    t = xht.reshape(N_GROUPS, 128, G, IN).transpose(0, 2, 1, 3)
    b = xhb.reshape(N_GROUPS, 96, G, IN).transpose(0, 2, 1, 3)
    return np.concatenate([t, b], axis=2).reshape(IMG_PER_CORE, IN, IN)


def _unshard_low(xlo):
    lo = xlo.reshape(N_GROUPS, LOW, G, LOW).transpose(0, 2, 1, 3)
    return lo.reshape(IMG_PER_CORE, LOW, LOW)


def run(x, T=None, t=None, trace=False):
    x = np.ascontiguousarray(np.asarray(x, dtype=np.float32))
    assert x.shape == (BATCH, CH, IN, IN), x.shape
    if T is None:
        T = dct_mat(8)
    if t is None:
        t = dct_mat(4)
    weights = build_weights(T, t)
    wmaps = {k: np.ascontiguousarray(v.astype(ml_dtypes.bfloat16))
             for k, v in weights.items()}

    per_core = BATCH // N_CORES  # 2
    in_maps = []
    for i in range(N_CORES):
        shard = x[i * per_core:(i + 1) * per_core].reshape(IMG_PER_CORE, IN, IN)
        xt, xb = _shard_layout(shard)
        in_maps.append({"xt": xt, "xb": xb, **wmaps})

    nc = _get_nc()
    res = run_bass_kernel_spmd(nc, in_maps, core_ids=list(range(N_CORES)),
                               trace=trace)
    xlow = np.stack([
        _unshard_low(np.asarray(res.results[i]["xlow"]))
        for i in range(N_CORES)
    ]).reshape(BATCH, CH, LOW, LOW)
    xhigh = np.stack([
        _unshard_high(np.asarray(res.results[i]["xht"]),
                      np.asarray(res.results[i]["xhb"]))
        for i in range(N_CORES)
    ]).reshape(BATCH, CH, IN, IN)
    return (xlow, xhigh), res


def kernel(x, T=None, t=None):
    (xlow, xhigh), _ = run(x, T, t, trace=False)
    return (xlow, xhigh)


if __name__ == "__main__":
    nc = build_nc()
    print("built ok")


# revision 9
# speedup vs baseline: 1.7547x; 1.1830x over previous
"""AsymDCT Trainium2 kernel.

Computes, for x (16, 64, 224, 224) f32:
  x_low  (16, 64, 112, 112) — 8x8 block DCT, keep low 4x4 coeffs, inverse with 4x4 DCT
  x_high (16, 64, 224, 224) — x minus the low-frequency reconstruction

Decomposition (per 224x224 image X, all matrices block-diagonal, exact):
  T4 = T[:4]  (4,8);  P8 = T4^T T4;  E = t^T T4
  phase1:  M2[c,r] = sum_k X[k,c] * BT4^T[k,r]        (stationary = X chunks)
  phase2:  [Q | L2] = M2^T @ [BP^T | BE^T]            (stationary = M2 chunks)
  phase3:  V = B' @ Q;  x_high = X - V   (B' = blockdiag(T4^T))
           x_low = Bt' @ L2              (Bt' = blockdiag(t^T))
PE compute in bf16 (fp32 PSUM accumulation); in/out f32; the x_high
subtraction reads the original f32 input.

Every matmul stationary is padded to 128 free columns (FWL eligibility);
the resulting junk output rows land in PSUM rows that are never copied out.

Data parallel: 1024 images sharded 128 per NeuronCore across 8 cores.
Host pre-permutes each shard into group-major layout (8 images per group)
so every device DMA is one fully contiguous 2D block.
"""

import sys

import numpy as np

for _p in ("/opt/trn_rl_repo",):
    if _p not in sys.path:
        sys.path.insert(0, _p)

import ml_dtypes  # noqa: E402

import concourse.bass as bass  # noqa: E402
import concourse.mybir as mybir  # noqa: E402
from concourse import bacc  # noqa: E402
from concourse import tile  # noqa: E402
from concourse.bass_utils import run_bass_kernel_spmd  # noqa: E402

DT = mybir.dt
F32 = DT.float32
BF16 = DT.bfloat16

N_CORES = 8
BATCH, CH, IN = 16, 64, 224
LOW = 112
IMG_PER_CORE = (BATCH // N_CORES) * CH  # 128
G = 8  # images per group
N_GROUPS = IMG_PER_CORE // G  # 16
NP = G // 2  # pairs per group

Alu = mybir.AluOpType


def dct_mat(N):
    n = np.arange(N)
    Tm = np.cos(np.pi * (2.0 * n[None, :] + 1.0) * n[:, None] / (2.0 * N))
    Tm[0, :] *= 1.0 / np.sqrt(2.0)
    Tm *= np.sqrt(2.0 / N)
    return Tm.astype(np.float32)


def _pad_cols(a, cols):
    out = np.zeros((a.shape[0], cols), dtype=a.dtype)
    out[:, :a.shape[1]] = a
    return out


def build_weights(T, t):
    """Host-side block-diagonal weight matrices (fp64; cast at feed time)."""
    T = np.asarray(T, dtype=np.float64)
    t = np.asarray(t, dtype=np.float64)
    T4 = T[:4, :]
    P8 = T4.T @ T4
    E = t.T @ T4
    W4 = np.kron(np.eye(28), T4)  # (112, 224)
    w2a = np.kron(np.eye(16), P8)          # (128, 128)
    w2b = np.kron(np.eye(12), P8)          # (96, 96)
    w3a = np.kron(np.eye(16), E).T         # (128, 64)
    w3b = np.kron(np.eye(12), E).T         # (96, 48)
    return {
        "w1a": np.kron(np.eye(16), T4).T,   # (128, 64)
        "w1b": np.kron(np.eye(12), T4).T,   # (96, 48)
        "w23a": np.concatenate([w2a, w3a], axis=1),  # (128, 192)
        "w23b": np.concatenate([w2b, w3b], axis=1),  # (96, 144)
        "w4a": W4[:, :128],                          # (112, 128)
        "w4b": _pad_cols(W4[:, 128:], 128),          # (112, 128) zero-padded
        "w5": _pad_cols(np.kron(np.eye(28), t), 128),  # (112, 128) zero-padded
    }


W_SHAPES = {
    "w1a": (128, 64), "w1b": (96, 48),
    "w23a": (128, 192), "w23b": (96, 144),
    "w4a": (112, 128), "w4b": (112, 128),
    "w5": (112, 128),
}

PAD = 32  # extra garbage columns on x/m2 tiles so M-padded lhsT slices stay in-bounds


def build_nc():
    nc = bacc.Bacc("TRN2", target_bir_lowering=False, debug=False,
                   num_devices=N_CORES)

    # Group-major host-prepared layouts: each DMA is one contiguous block.
    xin_t = nc.declare_dram_parameter("xt", [N_GROUPS, 128, G * IN], F32,
                                      isOutput=False)
    xin_b = nc.declare_dram_parameter("xb", [N_GROUPS, 96, G * IN], F32,
                                      isOutput=False)
    wd = {
        k: nc.declare_dram_parameter(k, list(sh), BF16, isOutput=False)
        for k, sh in W_SHAPES.items()
    }
    xlow_d = nc.declare_dram_parameter("xlow", [N_GROUPS, LOW, G * LOW], F32,
                                       isOutput=True)
    xhigh_t = nc.declare_dram_parameter("xht", [N_GROUPS, 128, G * IN], F32,
                                        isOutput=True)
    xhigh_b = nc.declare_dram_parameter("xhb", [N_GROUPS, 96, G * IN], F32,
                                        isOutput=True)

    with tile.TileContext(nc) as tc:
        with (
            tc.tile_pool(name="w", bufs=1) as wpool,
            tc.tile_pool(name="xf", bufs=3) as xfpool,
            tc.tile_pool(name="xi", bufs=2) as xpool,
            tc.tile_pool(name="m2", bufs=2) as m2pool,
            tc.tile_pool(name="qs", bufs=2) as qspool,
            tc.tile_pool(name="xh", bufs=2) as xhpool,
            tc.tile_pool(name="xls", bufs=2) as xlspool,
            tc.tile_pool(name="pp1", bufs=1, space="PSUM") as pp1,
            tc.tile_pool(name="ppq", bufs=1, space="PSUM") as ppq,
            tc.tile_pool(name="ppv", bufs=1, space="PSUM") as ppv,
            tc.tile_pool(name="ppxl", bufs=1, space="PSUM") as ppxl,
        ):
            w = {}
            for k, sh in W_SHAPES.items():
                w[k] = wpool.tile(list(sh), BF16, tag=k, name=k)
                nc.sync.dma_start(w[k][:], wd[k][:])

            for g in range(N_GROUPS):
                # ---- load f32 (HWDGE, contiguous), cast to bf16 on DVE ----
                xtf = xfpool.tile([128, G * IN], F32, tag="xtf")
                xbf = xfpool.tile([96, G * IN], F32, tag="xbf")
                nc.sync.dma_start(xtf[:], xin_t[g])
                nc.sync.dma_start(xbf[:], xin_b[g])
                xt = xpool.tile([128, G * IN + PAD], BF16, tag="xt")
                xb = xpool.tile([96, G * IN + PAD], BF16, tag="xb")
                nc.vector.tensor_copy(xt[:, :G * IN], xtf[:])
                nc.vector.tensor_copy(xb[:, :G * IN], xbf[:])

                m2t = m2pool.tile([128, G * LOW + PAD], BF16, tag="m2t")
                m2b = m2pool.tile([96, G * LOW + PAD], BF16, tag="m2b")
                qsa = qspool.tile([112, G * 192], BF16, tag="qsa")
                qsb = qspool.tile([112, G * 144], BF16, tag="qsb")
                xht = xhpool.tile([128, G * IN], F32, tag="xht")
                xhb = xhpool.tile([96, G * IN], F32, tag="xhb")
                xls = xlspool.tile([112, G * LOW], F32, tag="xls")

                for pp in range(NP):
                    # ---- phase 1 (pair): M2 pair block in PSUM ----
                    p1t = pp1.tile([128, 2 * LOW], F32, tag="p1t", bufs=2)
                    p1b = pp1.tile([128, 2 * LOW], F32, tag="p1b")
                    for jj in range(2):
                        j = pp * 2 + jj
                        xc = j * IN
                        mc = jj * LOW
                        nc.tensor.matmul(p1t[:, mc:mc + 64],
                                         xt[:, xc:xc + 128], w["w1a"][:])
                        nc.tensor.matmul(p1b[:, mc:mc + 64],
                                         xt[:, xc + 128:xc + 256], w["w1a"][:])
                        nc.tensor.matmul(p1t[:, mc + 64:mc + 112],
                                         xb[:, xc:xc + 128], w["w1b"][:])
                        nc.tensor.matmul(p1b[:, mc + 64:mc + 112],
                                         xb[:, xc + 128:xc + 256], w["w1b"][:])
                    h = pp * 2 * LOW
                    nc.scalar.copy(m2t[:, h:h + 2 * LOW], p1t[:])
                    nc.scalar.copy(m2b[:96, h:h + 2 * LOW], p1b[:96, :])

                    # ---- phase 2 (pair): [Q | L2] fused matmuls ----
                    qla = ppq.tile([128, 2 * 192], F32, tag="qla")
                    qlb = ppq.tile([128, 2 * 144], F32, tag="qlb")
                    for jj in range(2):
                        mc = (pp * 2 + jj) * LOW
                        nc.tensor.matmul(qla[:, jj * 192:(jj + 1) * 192],
                                         m2t[:, mc:mc + 128], w["w23a"][:])
                        nc.tensor.matmul(qlb[:, jj * 144:(jj + 1) * 144],
                                         m2b[:96, mc:mc + 128], w["w23b"][:])
                    nc.scalar.copy(qsa[:, pp * 384:(pp + 1) * 384],
                                   qla[:112, :])
                    nc.scalar.copy(qsb[:, pp * 288:(pp + 1) * 288],
                                   qlb[:112, :])

                    # ---- phase 3 high (pair): V = B' @ Q; x_high = X - V ----
                    vt = ppv.tile([128, 2 * IN], F32, tag="vt")
                    vb = ppv.tile([128, 2 * IN], F32, tag="vb")
                    rqa = qsa[:, pp * 384:(pp + 1) * 384].rearrange(
                        "p (i c) -> p i c", i=2)[:, :, 0:128]
                    rqb = qsb[:, pp * 288:(pp + 1) * 288].rearrange(
                        "p (i c) -> p i c", i=2)[:, :, 0:96]
                    vt3 = vt[:].rearrange("p (i c) -> p i c", i=2)
                    vb3 = vb[:].rearrange("p (i c) -> p i c", i=2)
                    nc.tensor.matmul(vt3[:, :, 0:128], w["w4a"][:], rqa)
                    nc.tensor.matmul(vt3[:, :, 128:224], w["w4a"][:], rqb)
                    nc.tensor.matmul(vb3[:, :, 0:128], w["w4b"][:], rqa)
                    nc.tensor.matmul(vb3[:, :, 128:224], w["w4b"][:], rqb)
                    pq = pp * 2 * IN
                    nc.vector.scalar_tensor_tensor(
                        xht[:, pq:pq + 2 * IN], xtf[:, pq:pq + 2 * IN], 1.0,
                        vt[:], Alu.mult, Alu.subtract)
                    nc.vector.scalar_tensor_tensor(
                        xhb[:, pq:pq + 2 * IN], xbf[:, pq:pq + 2 * IN], 1.0,
                        vb[:96, :], Alu.mult, Alu.subtract)

                    # ---- phase 3 low (pair): x_low = Bt' @ L2 ----
                    xlp = ppxl.tile([128, 2 * LOW], F32, tag="xlp")
                    la = qsa[:, pp * 384:(pp + 1) * 384].rearrange(
                        "p (i c) -> p i c", i=2)[:, :, 128:192]
                    lb = qsb[:, pp * 288:(pp + 1) * 288].rearrange(
                        "p (i c) -> p i c", i=2)[:, :, 96:144]
                    xlp3 = xlp[:].rearrange("p (i c) -> p i c", i=2)
                    nc.tensor.matmul(xlp3[:, :, 0:64], w["w5"][:], la)
                    nc.tensor.matmul(xlp3[:, :, 64:112], w["w5"][:], lb)
                    nc.scalar.copy(xls[:, h:h + 2 * LOW], xlp[:112, :])

                # ---- stores (contiguous blocks) ----
                nc.gpsimd.dma_start(xhigh_t[g], xht[:])
                nc.gpsimd.dma_start(xhigh_b[g], xhb[:])
                nc.sync.dma_start(xlow_d[g], xls[:])

    nc.compile()
    return nc


_NC_CACHE = None


def _get_nc():
    global _NC_CACHE
    if _NC_CACHE is None:
        _NC_CACHE = build_nc()
    return _NC_CACHE


def _shard_layout(shard):
    """(128, 224, 224) f32 -> group-major contiguous blocks."""
    grp = shard.reshape(N_GROUPS, G, IN, IN)
    xt = np.ascontiguousarray(
        grp[:, :, 0:128, :].transpose(0, 2, 1, 3)).reshape(N_GROUPS, 128,
                                                           G * IN)
    xb = np.ascontiguousarray(
        grp[:, :, 128:224, :].transpose(0, 2, 1, 3)).reshape(N_GROUPS, 96,
                                                             G * IN)
    return xt, xb


def _unshard_high(xht, xhb):
    """Inverse of _shard_layout for the x_high output."""
    t = xht.reshape(N_GROUPS, 128, G, IN).transpose(0, 2, 1, 3)
    b = xhb.reshape(N_GROUPS, 96, G, IN).transpose(0, 2, 1, 3)
    return np.concatenate([t, b], axis=2).reshape(IMG_PER_CORE, IN, IN)


def _unshard_low(xlo):
    lo = xlo.reshape(N_GROUPS, LOW, G, LOW).transpose(0, 2, 1, 3)
    return lo.reshape(IMG_PER_CORE, LOW, LOW)


def run(x, T=None, t=None, trace=False):
    x = np.ascontiguousarray(np.asarray(x, dtype=np.float32))
    assert x.shape == (BATCH, CH, IN, IN), x.shape
    if T is None:
        T = dct_mat(8)
    if t is None:
        t = dct_mat(4)
    weights = build_weights(T, t)
    wmaps = {k: np.ascontiguousarray(v.astype(ml_dtypes.bfloat16))
             for k, v in weights.items()}

    per_core = BATCH // N_CORES  # 2
    in_maps = []
    for i in range(N_CORES):
        shard = x[i * per_core:(i + 1) * per_core].reshape(IMG_PER_CORE, IN, IN)
        xt, xb = _shard_layout(shard)
        in_maps.append({"xt": xt, "xb": xb, **wmaps})

    nc = _get_nc()
    res = run_bass_kernel_spmd(nc, in_maps, core_ids=list(range(N_CORES)),
                               trace=trace)
    xlow = np.stack([
        _unshard_low(np.asarray(res.results[i]["xlow"]))
        for i in range(N_CORES)
    ]).reshape(BATCH, CH, LOW, LOW)
    xhigh = np.stack([
        _unshard_high(np.asarray(res.results[i]["xht"]),
                      np.asarray(res.results[i]["xhb"]))
        for i in range(N_CORES)
    ]).reshape(BATCH, CH, IN, IN)
    return (xlow, xhigh), res


def kernel(x, T=None, t=None):
    (xlow, xhigh), _ = run(x, T, t, trace=False)
    return (xlow, xhigh)


if __name__ == "__main__":
    nc = build_nc()
    print("built ok")
